# revision 68
# baseline (speedup 1.0000x reference)
"""GCN 2-layer + linear head on 8 Trainium2 NeuronCores (Bass/Tile).

v2 strategy:
- Phase A is REPLICATED: every core computes hs1 = dinv*(x@W1) for ALL
  100k nodes from a shared fp16 x^T input, so layer-1 needs no
  collective at all (the old version did 4 AllGathers per layer).
- Tables are PACKED 128-byte rows ([N, 64] fp16). dma_gather requires
  256B-multiple elements, so gathers fetch PAIRS of rows (idx = src//2)
  and edge tiles are sorted parity-pure so each tile's matmul reads the
  correct 64-feature half of the pair.
- Self-loops ride in the edge stream as ordinary edges, which keeps one
  global table layout for both layers and makes the epilogue a pure
  scale+relu.
- Aggregation is node-major: lhsT = one-hot S [128e, 112d], rhs =
  msg[128e, 64f] -> psum [112d, 64f]; 64-column matmuls.
- ONE AllGather [100000, 64] fp16 between the layers (cost-model:
  15us + 12.8MB @ ~54GB/s = 251us, vs 8 x 175us before).

SPMD: all 8 cores run one identical program; per-core differences live
only in input data (edge indices, dstrel, dinv columns). Tile counts are
padded to the max across cores.
"""

import numpy as np

import concourse.bacc as bacc
import concourse.mybir as mybir
import concourse.tile as tile
from concourse.bass_utils import run_bass_kernel_spmd

# problem shapes (hardcoded per contract)
N = 100000
E = 1600000
FIN = 128
HID = 64

NC_ = 8
P = 128
NLOC = N // NC_            # 12500 nodes per core
NCHUNK = 2                 # 25000-row chunks; pair idx < 12500 fits int16
CHUNK = N // NCHUNK
BS = 105                   # dst nodes per aggregation block
NBLK = (NLOC + BS - 1) // BS   # 120
SBB = 6                    # blocks per superblock (gather/S granularity)
NSB = (NBLK + SBB - 1) // SBB  # 28
GR = 8                     # node-tiles per phase-A group
NGC = (CHUNK + GR * P - 1) // (GR * P)  # phase-A groups per chunk (49)
CHPAD = NGC * GR * P       # chunk rows padded to whole groups (50176)
NG = NGC * NCHUNK          # 98 groups
NPAD = NG * GR * P
NLB = NBLK * BS            # 12544
SENT = 200.0               # dstrel sentinel for padded edge rows
QUAD = True                # quad-interleaved phase-A writes (512B descs)


# ----------------------------------------------------------------- host prep
def _prep(x, edge_index):
    x = np.asarray(x, np.float32)
    src_g = np.asarray(edge_index[0], np.int64)
    dst_g = np.asarray(edge_index[1], np.int64)

    deg = np.bincount(dst_g, minlength=N).astype(np.float32) + 1.0
    dinv = (1.0 / np.sqrt(deg)).astype(np.float32)

    # Table-row permutation: row(s) groups each core's half-shards so that
    # chunk c of the table equals AllGather(own rows [c*6250,(c+1)*6250)).
    half = NLOC // NCHUNK
    s_all = np.arange(N, dtype=np.int64)
    row_of = (((s_all % NLOC) // half) * CHUNK
              + (s_all // NLOC) * half + (s_all % NLOC) % half)
    s_of_row = np.empty(N, np.int64)
    s_of_row[row_of] = s_all

    # shared phase-A input: x^T fp16 in permuted row order. Each chunk of
    # table rows is padded to CHPAD (a whole number of phase-A groups) so the
    # chunk tables are separate DRAM tensors and layer-1 gathers can start as
    # soon as chunk 0 alone is written. Columns are additionally
    # quad-interleaved within 512-row units so the on-device [128, 4, 64]
    # tiles write 512B-contiguous quad rows.
    j = np.arange(NPAD, dtype=np.int64)
    if QUAD:
        colorder = (j // 512) * 512 + 4 * (j % 128) + (j % 512) // 128
    else:
        colorder = j
    rpos = np.arange(N, dtype=np.int64)
    rpos += (rpos // CHUNK) * (CHPAD - CHUNK)   # table row -> padded position
    # dinv is folded into x on the host: hs1 = dinv*(x@W1) = (dinv*x)@W1
    xpad = np.zeros((NPAD, FIN), np.float16)
    xpad[rpos] = x[s_of_row] * dinv[s_of_row][:, None]
    xT = np.ascontiguousarray(xpad[colorder].T)

    # per-core edge streams -------------------------------------------------
    # cells: (block, chunk); edges sorted even-parity-first inside the cell,
    # padded ONCE per cell. Tiles in the even/odd boundary band get TWO
    # matmul instances (even- and odd-half of the gathered pair).
    loop = np.arange(N, dtype=np.int64)
    cores_raw = []
    ncell = NBLK * NCHUNK
    e_cnt = np.zeros((NC_, ncell), np.int64)   # even-parity edges per cell
    t_cnt = np.zeros((NC_, ncell), np.int64)   # total edges per cell
    for k in range(NC_):
        m = (dst_g // NLOC) == k
        s = np.concatenate([src_g[m], loop[k * NLOC:(k + 1) * NLOC]])
        d = np.concatenate([dst_g[m], loop[k * NLOC:(k + 1) * NLOC]]) - k * NLOC
        b = d // BS
        r = row_of[s]                              # permuted table row
        c = r // CHUNK
        par = r % 2
        il = ((r % CHUNK) // 2).astype(np.int16)   # pair index in chunk
        key = b * NCHUNK + c
        o = np.lexsort((par, key))
        t_cnt[k] = np.bincount(key, minlength=ncell)
        e_cnt[k] = np.bincount(key[par == 0], minlength=ncell)
        cores_raw.append((il[o], (d % BS)[o], par[o],
                          np.concatenate([[0], np.cumsum(t_cnt[k])])))

    nt = np.maximum((t_cnt.max(axis=0) + P - 1) // P, 1)       # gather tiles
    e_hi = (e_cnt.max(axis=0) + P - 1) // P                    # even tiles
    o_lo = np.where((t_cnt - e_cnt).max(axis=0) > 0,
                    e_cnt.min(axis=0) // P, nt)                # 1st odd tile
    o_hi = np.where((t_cnt - e_cnt).max(axis=0) > 0,
                    (t_cnt.max(axis=0) + P - 1) // P, 0)
    nt = nt.reshape(NBLK, NCHUNK)
    e_hi = e_hi.reshape(NBLK, NCHUNK)
    o_lo = o_lo.reshape(NBLK, NCHUNK)
    o_hi = o_hi.reshape(NBLK, NCHUNK)
    # instances per cell: even tiles [0, e_hi), odd tiles [o_lo, o_hi)
    ni = e_hi + np.maximum(o_hi - o_lo, 0)

    # per-(superblock, chunk): gather tiles and S columns
    seg_tiles = np.array([[nt[sb * SBB:min((sb + 1) * SBB, NBLK), c].sum()
                           for c in range(NCHUNK)] for sb in range(NSB)])
    seg_scols = np.array([[ni[sb * SBB:min((sb + 1) * SBB, NBLK), c].sum()
                           for c in range(NCHUNK)] for sb in range(NSB)])
    st_max = int(seg_tiles.max())
    sc_max = int(seg_scols.max())
    tt = int(seg_tiles.sum())          # total gather tiles per layer
    tts = int(seg_scols.sum())         # total S columns per layer

    def cell_instances(b, c):
        out = []
        for t in range(int(e_hi[b, c])):
            out.append((t, 0))
        for t in range(int(o_lo[b, c]), int(o_hi[b, c])):
            out.append((t, 1))
        return out

    # matmul schedule per block: (chunk, tile-in-(sb,c), scol-in-(sb,c), par)
    mms_all = []
    for b in range(NBLK):
        blk_lo = (b // SBB) * SBB
        mms = []
        for c in range(NCHUNK):
            t_off = int(nt[blk_lo:b, c].sum())
            s_off = int(ni[blk_lo:b, c].sum())
            for i, (t, par) in enumerate(cell_instances(b, c)):
                mms.append((c, t_off + t, s_off + i, par))
        mms_all.append(mms)

    in_maps = []
    for k in range(NC_):
        il_s, dr_s, par_s, cum = cores_raw[k]
        idx_cols, dst_cols = [], []
        for sb in range(NSB):
            blk_lo, blk_hi = sb * SBB, min((sb + 1) * SBB, NBLK)
            for c in range(NCHUNK):
                ils = []
                for b in range(blk_lo, blk_hi):
                    g = b * NCHUNK + c
                    a0, a1 = cum[g], cum[g + 1]
                    npad = int(nt[b, c] * P - (a1 - a0))
                    ils.append(il_s[a0:a1])
                    ils.append(np.zeros(npad, np.int16))
                    # dstrel columns, one per matmul instance
                    drs = np.concatenate([dr_s[a0:a1],
                                          np.full(npad, SENT, np.int64)])
                    prs = np.concatenate([par_s[a0:a1],
                                          np.full(npad, 2, np.int64)])
                    for t, par in cell_instances(b, c):
                        seg = drs[t * P:(t + 1) * P]
                        pseg = prs[t * P:(t + 1) * P]
                        dst_cols.append(
                            np.where(pseg == par, seg, SENT)
                            .astype(np.float16)[:, None])
                seg_il = np.concatenate(ils)
                wrapped = seg_il.reshape(-1, 16).T          # [16, n/16]
                idx_cols.append(np.tile(wrapped, (8, 1)))   # [128, n/16]

        dinvB = np.zeros((P, NBLK), np.float32)
        dloc = dinv[k * NLOC:(k + 1) * NLOC]
        for b in range(NBLK):
            w = min(BS, NLOC - b * BS)
            dinvB[:w, b] = dloc[b * BS:b * BS + w]

        in_maps.append({
            "xT": xT,
            "idx": np.concatenate(idx_cols, axis=1),        # [128, tt*8] i16
            "dstrel": np.concatenate(dst_cols, axis=1),     # [128, tt] fp16
            "dinvB": dinvB,
        })

    iota_w = np.tile(np.arange(P, dtype=np.float16), (P, 1))
    eye = np.eye(P, dtype=np.float16)
    for m in in_maps:
        m["iota"] = iota_w
        m["eye"] = eye
    return (in_maps, nt, seg_tiles, seg_scols, st_max, sc_max, tt, tts,
            mms_all)


# ------------------------------------------------------------- device build
def _build(seg_tiles, seg_scols, st_max, sc_max, tt, tts, mms_all,
           has_b1, has_b2, has_bc):
    f32, f16, i16 = mybir.dt.float32, mybir.dt.float16, mybir.dt.int16
    nc = bacc.Bacc("TRN2", num_devices=NC_)

    xT = nc.dram_tensor("xT", [FIN, NPAD], f16, kind="ExternalInput")
    idx = nc.dram_tensor("idx", [P, tt * 8], i16, kind="ExternalInput")
    dstrel = nc.dram_tensor("dstrel", [P, tts], f16, kind="ExternalInput")
    iota = nc.dram_tensor("iota", [P, P], f16, kind="ExternalInput")
    eye = nc.dram_tensor("eye", [P, P], f16, kind="ExternalInput")
    dinvB = nc.dram_tensor("dinvB", [P, NBLK], f32, kind="ExternalInput")
    W1 = nc.dram_tensor("W1", [FIN, HID], f32, kind="ExternalInput")
    W2 = nc.dram_tensor("W2", [HID, HID], f32, kind="ExternalInput")
    WcBC = nc.dram_tensor("WcBC", [P, HID], f16, kind="ExternalInput")
    b1bc = nc.dram_tensor("b1bc", [P, HID], f32, kind="ExternalInput")
    b2bc = nc.dram_tensor("b2bc", [P, HID], f32, kind="ExternalInput")
    bc = nc.dram_tensor("bc", [1, 1], f32, kind="ExternalInput")
    out = nc.dram_tensor("out", [NLB, 1], f32, kind="ExternalOutput")

    relu = mybir.ActivationFunctionType.Relu
    copy_ = mybir.ActivationFunctionType.Copy

    with tile.TileContext(nc) as tc:
        with (
            tc.tile_pool(name="cst", bufs=1) as cst,
            tc.tile_pool(name="io", bufs=8) as io,
            tc.tile_pool(name="msgp", bufs=5) as msgp,
            tc.tile_pool(name="sp", bufs=5) as sp,
            tc.tile_pool(name="work", bufs=10) as work,
            tc.tile_pool(name="accp", bufs=1) as accp,
            tc.tile_pool(name="psA", bufs=2, space="PSUM") as psA,
            tc.tile_pool(name="agg", bufs=3, space="PSUM") as aggp,
            tc.tile_pool(name="ptr", bufs=1, space="PSUM") as ptrp,
            tc.tile_pool(name="p2", bufs=2, space="PSUM") as p2p,
            tc.tile_pool(name="dram", bufs=1, space="DRAM") as dram,
        ):
            # constants
            W1sb = cst.tile([FIN, HID], f32)
            nc.sync.dma_start(W1sb[:], W1[:])
            W1h = cst.tile([FIN, HID], f16)
            nc.vector.tensor_copy(out=W1h[:], in_=W1sb[:])
            W2sb = cst.tile([HID, HID], f32)
            nc.sync.dma_start(W2sb[:], W2[:])
            W2h = cst.tile([HID, HID], f16)
            nc.vector.tensor_copy(out=W2h[:], in_=W2sb[:])
            Wcb = cst.tile([P, HID], f16)
            nc.sync.dma_start(Wcb[:], WcBC[:])
            eyesb = cst.tile([P, P], f16)
            nc.sync.dma_start(eyesb[:], eye[:])
            iotasb = cst.tile([P, P], f16)
            nc.sync.dma_start(iotasb[:], iota[:])
            dst_sb = cst.tile([P, tts], f16)
            nc.sync.dma_start(dst_sb[:], dstrel[:])
            dBsb = cst.tile([P, NBLK], f32)
            nc.sync.dma_start(dBsb[:], dinvB[:])
            if has_b1:
                b1sb = cst.tile([P, HID], f32)
                nc.sync.dma_start(b1sb[:], b1bc[:])
            if has_b2:
                b2sb = cst.tile([P, HID], f32)
                nc.sync.dma_start(b2sb[:], b2bc[:])
            if has_bc:
                bcsb = cst.tile([1, 1], f32)
                nc.sync.dma_start(bcsb[:], bc[:])

            hs1f = [dram.tile([CHPAD, HID], f16, name=f"hs1f{c}")
                    for c in range(NCHUNK)]
            hs2s = dram.tile([NLB, HID], f16)
            hs2f = [dram.tile([CHUNK, HID], f16, addr_space="Shared",
                              name=f"hs2f{c}") for c in range(NCHUNK)]

            # ---- phase A: hs1 = dinv * (x @ W1) for ALL nodes, fp16 packed
            for g in range(NG):
                xb = io.tile([FIN, GR * P], f16, name="xb")
                nc.sync.dma_start(xb[:], xT[:, g * GR * P:(g + 1) * GR * P])
                ps = psA.tile([P, GR, HID], f32, name="psA")
                for j in range(GR):
                    nc.tensor.matmul(out=ps[:, j, :],
                                     lhsT=xb[:, j * P:(j + 1) * P],
                                     rhs=W1h[:], start=True, stop=True)
                hsg = work.tile([P, GR, HID], f16, name="hsg")
                nc.scalar.activation(out=hsg[:], in_=ps[:], func=copy_)
                gl = (g % NGC) * GR * P
                dst1 = hs1f[g // NGC][gl:gl + GR * P, :]
                if QUAD:
                    nc.sync.dma_start(
                        dst1.rearrange("(u p r) f -> p u (r f)", u=2, p=P),
                        hsg[:].rearrange("p (u q) f -> p u (q f)", u=2))
                else:
                    nc.sync.dma_start(
                        dst1.rearrange("(j p) f -> p j f", p=P),
                        hsg[:])

            # stream column/idx offsets per (sb, chunk)
            col_off = np.zeros((NSB, NCHUNK), np.int64)
            ixw_off = np.zeros((NSB, NCHUNK), np.int64)
            acc_to, acc_ti = 0, 0
            for sb in range(NSB):
                for c in range(NCHUNK):
                    col_off[sb, c] = acc_to
                    ixw_off[sb, c] = acc_ti
                    acc_to += int(seg_scols[sb][c])
                    acc_ti += int(seg_tiles[sb][c]) * P // 16

            def seg_load(table, sb, c):
                st = int(seg_tiles[sb][c])
                sc = int(seg_scols[sb][c])
                n_idx = st * P
                io_ = int(ixw_off[sb, c])
                to_ = int(col_off[sb, c])
                ix = io.tile([P, n_idx // 16], i16, name="ix")
                nc.sync.dma_start(ix[:], idx[:, io_:io_ + n_idx // 16])
                mg = msgp.tile([P, st, P], f16, name="mg")
                nc.gpsimd.dma_gather(
                    mg[:],
                    table.rearrange("(q two) f -> q (two f)", two=2),
                    ix[:], n_idx, n_idx, P, single_packet=False)
                St = sp.tile([P, sc, BS], f16, name="St")
                nc.vector.tensor_tensor(
                    out=St[:],
                    in0=dst_sb[:, to_:to_ + sc, None]
                        .to_broadcast([P, sc, BS]),
                    in1=iotasb[:, None, :BS]
                        .to_broadcast([P, sc, BS]),
                    op=mybir.AluOpType.is_equal)
                return mg, St

            def block_mms(pt, b, msgs, Ss, chunks, start, stop):
                mms = [e for e in mms_all[b] if e[0] in chunks]
                for j, (c, t, sc, par) in enumerate(mms):
                    nc.tensor.matmul(
                        out=pt[:],
                        lhsT=Ss[c][:, sc, :],
                        rhs=msgs[c][:, t, par * HID:(par + 1) * HID],
                        start=start and (j == 0),
                        stop=stop and (j == len(mms) - 1))

            def scale_relu(pt_or_t, b, has_b, bsb):
                """h = relu(dinv*x + b) as fp16 [BS, HID]."""
                hL = work.tile([BS, HID], f16, name="hL")
                if has_b:
                    tsc = work.tile([BS, HID], f32, name="tsc")
                    nc.scalar.activation(out=tsc[:], in_=pt_or_t[:],
                                         func=copy_,
                                         scale=dBsb[:BS, b:b + 1])
                    tbb = work.tile([BS, HID], f32, name="tbb")
                    nc.vector.tensor_tensor(out=tbb[:], in0=tsc[:],
                                            in1=bsb[:BS, :],
                                            op=mybir.AluOpType.add)
                    nc.scalar.activation(out=hL[:], in_=tbb[:], func=relu)
                else:
                    nc.scalar.activation(out=hL[:], in_=pt_or_t[:],
                                         func=relu,
                                         scale=dBsb[:BS, b:b + 1])
                return hL

            half = NLOC // NCHUNK
            ag_after_sb = {}
            for c in range(NCHUNK - 1):
                # AG#c ins ready once blocks covering rows < (c+1)*half done
                blk_ready = ((c + 1) * half + BS - 1) // BS
                ag_after_sb[min((blk_ready - 1) // SBB + 1, NSB - 1)] = c

            # ---- layer 1: one pass over both chunks, epilogue -> hs2s
            for sb in range(NSB):
                blk_lo, blk_hi = sb * SBB, min((sb + 1) * SBB, NBLK)
                msgs, Ss = [], []
                for c in range(NCHUNK):
                    mg, St = seg_load(hs1f[c][0:CHUNK, :], sb, c)
                    msgs.append(mg)
                    Ss.append(St)
                for b in range(blk_lo, blk_hi):
                    pt = aggp.tile([BS, HID], f32, name="pt")
                    block_mms(pt, b, msgs, Ss, tuple(range(NCHUNK)), True, True)
                    hL = scale_relu(pt, b, has_b1, b1sb if has_b1 else None)
                    # hs2 = dinv * (h1 @ W2): transpose h1 then matmul
                    ptr = ptrp.tile([HID, BS], f16, name="ptr")
                    nc.tensor.transpose(ptr[:], hL[:], eyesb[:BS, :BS])
                    h1T = work.tile([HID, BS], f16, name="h1T")
                    nc.scalar.activation(out=h1T[:], in_=ptr[:], func=copy_)
                    ps2 = p2p.tile([BS, HID], f32, name="ps2")
                    nc.tensor.matmul(out=ps2[:], lhsT=h1T[:],
                                     rhs=W2h[:], start=True, stop=True)
                    h2r = work.tile([BS, HID], f16, name="h2r")
                    nc.scalar.activation(out=h2r[:], in_=ps2[:],
                                         func=copy_,
                                         scale=dBsb[:BS, b:b + 1])
                    nc.sync.dma_start(hs2s[b * BS:(b + 1) * BS, :], h2r[:])
                if sb in ag_after_sb:
                    c = ag_after_sb[sb]
                    nc.gpsimd.collective_compute(
                        "AllGather", mybir.AluOpType.bypass,
                        replica_groups=[list(range(NC_))],
                        ins=[hs2s[c * half:(c + 1) * half, :]],
                        outs=[hs2f[c][:]],
                    )
            nc.gpsimd.collective_compute(
                "AllGather", mybir.AluOpType.bypass,
                replica_groups=[list(range(NC_))],
                ins=[hs2s[(NCHUNK - 1) * half:NCHUNK * half, :]],
                outs=[hs2f[NCHUNK - 1][:]],
            )

            # ---- layer 2: one pass per chunk, partials held in SBUF accs
            # (ping-pong buffers: pass p writes acc[p%2] reading acc[1-p%2])
            accs = [accp.tile([P, NBLK, HID], f16, name=f"acc{i}")
                    for i in range(2)]
            for cpass in range(NCHUNK):
                last = cpass == NCHUNK - 1
                awr = accs[cpass % 2]
                ard = accs[1 - cpass % 2]
                for sb in range(NSB):
                    blk_lo, blk_hi = sb * SBB, min((sb + 1) * SBB, NBLK)
                    mg, St = seg_load(hs2f[cpass][:], sb, cpass)
                    for b in range(blk_lo, blk_hi):
                        pt = aggp.tile([BS, HID], f32, name="pt")
                        block_mms(pt, b, {cpass: mg}, {cpass: St},
                                  (cpass,), True, True)
                        if not last:
                            if cpass == 0:
                                nc.scalar.activation(out=awr[:BS, b, :],
                                                     in_=pt[:], func=copy_)
                            else:
                                nc.vector.tensor_tensor(
                                    out=awr[:BS, b, :], in0=pt[:],
                                    in1=ard[:BS, b, :],
                                    op=mybir.AluOpType.add)
                            continue
                        t2 = work.tile([BS, HID], f32, name="t2")
                        nc.vector.tensor_tensor(out=t2[:], in0=pt[:],
                                                in1=ard[:BS, b, :],
                                                op=mybir.AluOpType.add)
                        hL = scale_relu(t2, b, has_b2,
                                        b2sb if has_b2 else None)
                        # head: out = relu(h2) @ Wc (+ bc); the free-axis sum
                        # rides the Act accumulator, the mult runs on Pool
                        mwc = work.tile([BS, HID], f32, name="mwc")
                        nc.gpsimd.tensor_mul(mwc[:], hL[:], Wcb[:BS, :])
                        mwd = work.tile([BS, HID], f32, name="mwd")
                        oc = work.tile([BS, 1], f32, name="oc")
                        nc.scalar.activation(out=mwd[:], in_=mwc[:],
                                             func=copy_, accum_out=oc[:])
                        if has_bc:
                            oc2 = work.tile([BS, 1], f32, name="oc2")
                            nc.vector.tensor_scalar(
                                out=oc2[:], in0=oc[:],
                                scalar1=bcsb[:1, :1], scalar2=None,
                                op0=mybir.AluOpType.add)
                            oc = oc2
                        nc.sync.dma_start(
                            out[b * BS:(b + 1) * BS, :], oc[:])

    nc.compile()
    return nc


_CACHE = {}


def kernel(x, edge_index, W1, b1, W2, b2, Wc, bc):
    x = np.asarray(x, np.float32)
    edge_index = np.asarray(edge_index, np.int32)
    (in_maps, nt, seg_tiles, seg_scols, st_max, sc_max, tt, tts,
     mms_all) = _prep(x, edge_index)

    b1 = np.asarray(b1, np.float32).reshape(-1)
    b2 = np.asarray(b2, np.float32).reshape(-1)
    bc = np.asarray(bc, np.float32).reshape(-1)
    Wc = np.asarray(Wc, np.float32).reshape(-1)
    has_b1 = bool(np.any(b1 != 0))
    has_b2 = bool(np.any(b2 != 0))
    has_bc = bool(np.any(bc != 0))

    key = (st_max, tt, tts, nt.tobytes(),
           tuple(np.asarray(seg_scols).ravel()), has_b1, has_b2, has_bc)
    if key not in _CACHE:
        _CACHE[key] = _build(seg_tiles, seg_scols, st_max, sc_max, tt, tts,
                             mms_all, has_b1, has_b2, has_bc)
    nc = _CACHE[key]

    shared = {
        "W1": np.asarray(W1, np.float32),
        "W2": np.asarray(W2, np.float32),
        "WcBC": np.tile(Wc.astype(np.float16), (P, 1)),
        "b1bc": np.tile(b1, (P, 1)),
        "b2bc": np.tile(b2, (P, 1)),
        "bc": bc.reshape(1, 1),
    }
    for m in in_maps:
        m.update(shared)

    res = run_bass_kernel_spmd(nc, in_maps, core_ids=list(range(NC_)))
    return np.concatenate(
        [res.results[k]["out"][:NLOC, 0] for k in range(NC_)]
    ).astype(np.float32)


# revision 69
# speedup vs baseline: 1.0259x; 1.0259x over previous
"""GCN 2-layer + linear head on 8 Trainium2 NeuronCores (Bass/Tile).

v2 strategy:
- Phase A is REPLICATED: every core computes hs1 = dinv*(x@W1) for ALL
  100k nodes from a shared fp16 x^T input, so layer-1 needs no
  collective at all (the old version did 4 AllGathers per layer).
- Tables are PACKED 128-byte rows ([N, 64] fp16). dma_gather requires
  256B-multiple elements, so gathers fetch PAIRS of rows (idx = src//2)
  and edge tiles are sorted parity-pure so each tile's matmul reads the
  correct 64-feature half of the pair.
- Self-loops ride in the edge stream as ordinary edges, which keeps one
  global table layout for both layers and makes the epilogue a pure
  scale+relu.
- Aggregation is node-major: lhsT = one-hot S [128e, 112d], rhs =
  msg[128e, 64f] -> psum [112d, 64f]; 64-column matmuls.
- ONE AllGather [100000, 64] fp16 between the layers (cost-model:
  15us + 12.8MB @ ~54GB/s = 251us, vs 8 x 175us before).

SPMD: all 8 cores run one identical program; per-core differences live
only in input data (edge indices, dstrel, dinv columns). Tile counts are
padded to the max across cores.
"""

import numpy as np

import concourse.bacc as bacc
import concourse.mybir as mybir
import concourse.tile as tile
from concourse.bass_utils import run_bass_kernel_spmd

# problem shapes (hardcoded per contract)
N = 100000
E = 1600000
FIN = 128
HID = 64

NC_ = 8
P = 128
NLOC = N // NC_            # 12500 nodes per core
NCHUNK = 2                 # 25000-row chunks; pair idx < 12500 fits int16
CHUNK = N // NCHUNK
BS = 105                   # dst nodes per aggregation block
NBLK = (NLOC + BS - 1) // BS   # 120
SBB = 4                    # blocks per superblock (gather/S granularity)
NSB = (NBLK + SBB - 1) // SBB  # 28
GR = 8                     # node-tiles per phase-A group
NGC = (CHUNK + GR * P - 1) // (GR * P)  # phase-A groups per chunk (49)
CHPAD = NGC * GR * P       # chunk rows padded to whole groups (50176)
NG = NGC * NCHUNK          # 98 groups
NPAD = NG * GR * P
NLB = NBLK * BS            # 12544
SENT = 200.0               # dstrel sentinel for padded edge rows
QUAD = True                # quad-interleaved phase-A writes (512B descs)


# ----------------------------------------------------------------- host prep
def _prep(x, edge_index):
    x = np.asarray(x, np.float32)
    src_g = np.asarray(edge_index[0], np.int64)
    dst_g = np.asarray(edge_index[1], np.int64)

    deg = np.bincount(dst_g, minlength=N).astype(np.float32) + 1.0
    dinv = (1.0 / np.sqrt(deg)).astype(np.float32)

    # Table-row permutation: row(s) groups each core's half-shards so that
    # chunk c of the table equals AllGather(own rows [c*6250,(c+1)*6250)).
    half = NLOC // NCHUNK
    s_all = np.arange(N, dtype=np.int64)
    row_of = (((s_all % NLOC) // half) * CHUNK
              + (s_all // NLOC) * half + (s_all % NLOC) % half)
    s_of_row = np.empty(N, np.int64)
    s_of_row[row_of] = s_all

    # shared phase-A input: x^T fp16 in permuted row order. Each chunk of
    # table rows is padded to CHPAD (a whole number of phase-A groups) so the
    # chunk tables are separate DRAM tensors and layer-1 gathers can start as
    # soon as chunk 0 alone is written. Columns are additionally
    # quad-interleaved within 512-row units so the on-device [128, 4, 64]
    # tiles write 512B-contiguous quad rows.
    j = np.arange(NPAD, dtype=np.int64)
    if QUAD:
        colorder = (j // 512) * 512 + 4 * (j % 128) + (j % 512) // 128
    else:
        colorder = j
    rpos = np.arange(N, dtype=np.int64)
    rpos += (rpos // CHUNK) * (CHPAD - CHUNK)   # table row -> padded position
    # dinv is folded into x on the host: hs1 = dinv*(x@W1) = (dinv*x)@W1
    xpad = np.zeros((NPAD, FIN), np.float16)
    xpad[rpos] = x[s_of_row] * dinv[s_of_row][:, None]
    xT = np.ascontiguousarray(xpad[colorder].T)

    # per-core edge streams -------------------------------------------------
    # cells: (block, chunk); edges sorted even-parity-first inside the cell,
    # padded ONCE per cell. Tiles in the even/odd boundary band get TWO
    # matmul instances (even- and odd-half of the gathered pair).
    loop = np.arange(N, dtype=np.int64)
    cores_raw = []
    ncell = NBLK * NCHUNK
    e_cnt = np.zeros((NC_, ncell), np.int64)   # even-parity edges per cell
    t_cnt = np.zeros((NC_, ncell), np.int64)   # total edges per cell
    for k in range(NC_):
        m = (dst_g // NLOC) == k
        s = np.concatenate([src_g[m], loop[k * NLOC:(k + 1) * NLOC]])
        d = np.concatenate([dst_g[m], loop[k * NLOC:(k + 1) * NLOC]]) - k * NLOC
        b = d // BS
        r = row_of[s]                              # permuted table row
        c = r // CHUNK
        par = r % 2
        il = ((r % CHUNK) // 2).astype(np.int16)   # pair index in chunk
        key = b * NCHUNK + c
        o = np.lexsort((par, key))
        t_cnt[k] = np.bincount(key, minlength=ncell)
        e_cnt[k] = np.bincount(key[par == 0], minlength=ncell)
        cores_raw.append((il[o], (d % BS)[o], par[o],
                          np.concatenate([[0], np.cumsum(t_cnt[k])])))

    nt = np.maximum((t_cnt.max(axis=0) + P - 1) // P, 1)       # gather tiles
    e_hi = (e_cnt.max(axis=0) + P - 1) // P                    # even tiles
    o_lo = np.where((t_cnt - e_cnt).max(axis=0) > 0,
                    e_cnt.min(axis=0) // P, nt)                # 1st odd tile
    o_hi = np.where((t_cnt - e_cnt).max(axis=0) > 0,
                    (t_cnt.max(axis=0) + P - 1) // P, 0)
    nt = nt.reshape(NBLK, NCHUNK)
    e_hi = e_hi.reshape(NBLK, NCHUNK)
    o_lo = o_lo.reshape(NBLK, NCHUNK)
    o_hi = o_hi.reshape(NBLK, NCHUNK)
    # instances per cell: even tiles [0, e_hi), odd tiles [o_lo, o_hi)
    ni = e_hi + np.maximum(o_hi - o_lo, 0)

    # per-(superblock, chunk): gather tiles and S columns
    seg_tiles = np.array([[nt[sb * SBB:min((sb + 1) * SBB, NBLK), c].sum()
                           for c in range(NCHUNK)] for sb in range(NSB)])
    seg_scols = np.array([[ni[sb * SBB:min((sb + 1) * SBB, NBLK), c].sum()
                           for c in range(NCHUNK)] for sb in range(NSB)])
    st_max = int(seg_tiles.max())
    sc_max = int(seg_scols.max())
    tt = int(seg_tiles.sum())          # total gather tiles per layer
    tts = int(seg_scols.sum())         # total S columns per layer

    def cell_instances(b, c):
        out = []
        for t in range(int(e_hi[b, c])):
            out.append((t, 0))
        for t in range(int(o_lo[b, c]), int(o_hi[b, c])):
            out.append((t, 1))
        return out

    # matmul schedule per block: (chunk, tile-in-(sb,c), scol-in-(sb,c), par)
    mms_all = []
    for b in range(NBLK):
        blk_lo = (b // SBB) * SBB
        mms = []
        for c in range(NCHUNK):
            t_off = int(nt[blk_lo:b, c].sum())
            s_off = int(ni[blk_lo:b, c].sum())
            for i, (t, par) in enumerate(cell_instances(b, c)):
                mms.append((c, t_off + t, s_off + i, par))
        mms_all.append(mms)

    in_maps = []
    for k in range(NC_):
        il_s, dr_s, par_s, cum = cores_raw[k]
        idx_cols, dst_cols = [], []
        for sb in range(NSB):
            blk_lo, blk_hi = sb * SBB, min((sb + 1) * SBB, NBLK)
            for c in range(NCHUNK):
                ils = []
                for b in range(blk_lo, blk_hi):
                    g = b * NCHUNK + c
                    a0, a1 = cum[g], cum[g + 1]
                    npad = int(nt[b, c] * P - (a1 - a0))
                    ils.append(il_s[a0:a1])
                    ils.append(np.zeros(npad, np.int16))
                    # dstrel columns, one per matmul instance
                    drs = np.concatenate([dr_s[a0:a1],
                                          np.full(npad, SENT, np.int64)])
                    prs = np.concatenate([par_s[a0:a1],
                                          np.full(npad, 2, np.int64)])
                    for t, par in cell_instances(b, c):
                        seg = drs[t * P:(t + 1) * P]
                        pseg = prs[t * P:(t + 1) * P]
                        dst_cols.append(
                            np.where(pseg == par, seg, SENT)
                            .astype(np.float16)[:, None])
                seg_il = np.concatenate(ils)
                wrapped = seg_il.reshape(-1, 16).T          # [16, n/16]
                idx_cols.append(np.tile(wrapped, (8, 1)))   # [128, n/16]

        dinvB = np.zeros((P, NBLK), np.float32)
        dloc = dinv[k * NLOC:(k + 1) * NLOC]
        for b in range(NBLK):
            w = min(BS, NLOC - b * BS)
            dinvB[:w, b] = dloc[b * BS:b * BS + w]

        in_maps.append({
            "xT": xT,
            "idx": np.concatenate(idx_cols, axis=1),        # [128, tt*8] i16
            "dstrel": np.concatenate(dst_cols, axis=1),     # [128, tt] fp16
            "dinvB": dinvB,
        })

    iota_w = np.tile(np.arange(P, dtype=np.float16), (P, 1))
    eye = np.eye(P, dtype=np.float16)
    for m in in_maps:
        m["iota"] = iota_w
        m["eye"] = eye
    return (in_maps, nt, seg_tiles, seg_scols, st_max, sc_max, tt, tts,
            mms_all)


# ------------------------------------------------------------- device build
def _build(seg_tiles, seg_scols, st_max, sc_max, tt, tts, mms_all,
           has_b1, has_b2, has_bc):
    f32, f16, i16 = mybir.dt.float32, mybir.dt.float16, mybir.dt.int16
    nc = bacc.Bacc("TRN2", num_devices=NC_)

    xT = nc.dram_tensor("xT", [FIN, NPAD], f16, kind="ExternalInput")
    idx = nc.dram_tensor("idx", [P, tt * 8], i16, kind="ExternalInput")
    dstrel = nc.dram_tensor("dstrel", [P, tts], f16, kind="ExternalInput")
    iota = nc.dram_tensor("iota", [P, P], f16, kind="ExternalInput")
    eye = nc.dram_tensor("eye", [P, P], f16, kind="ExternalInput")
    dinvB = nc.dram_tensor("dinvB", [P, NBLK], f32, kind="ExternalInput")
    W1 = nc.dram_tensor("W1", [FIN, HID], f32, kind="ExternalInput")
    W2 = nc.dram_tensor("W2", [HID, HID], f32, kind="ExternalInput")
    WcBC = nc.dram_tensor("WcBC", [P, HID], f16, kind="ExternalInput")
    b1bc = nc.dram_tensor("b1bc", [P, HID], f32, kind="ExternalInput")
    b2bc = nc.dram_tensor("b2bc", [P, HID], f32, kind="ExternalInput")
    bc = nc.dram_tensor("bc", [1, 1], f32, kind="ExternalInput")
    out = nc.dram_tensor("out", [NLB, 1], f32, kind="ExternalOutput")

    relu = mybir.ActivationFunctionType.Relu
    copy_ = mybir.ActivationFunctionType.Copy

    with tile.TileContext(nc) as tc:
        with (
            tc.tile_pool(name="cst", bufs=1) as cst,
            tc.tile_pool(name="io", bufs=8) as io,
            tc.tile_pool(name="msgp", bufs=7) as msgp,
            tc.tile_pool(name="sp", bufs=7) as sp,
            tc.tile_pool(name="work", bufs=12) as work,
            tc.tile_pool(name="accp", bufs=1) as accp,
            tc.tile_pool(name="psA", bufs=2, space="PSUM") as psA,
            tc.tile_pool(name="agg", bufs=3, space="PSUM") as aggp,
            tc.tile_pool(name="ptr", bufs=1, space="PSUM") as ptrp,
            tc.tile_pool(name="p2", bufs=2, space="PSUM") as p2p,
            tc.tile_pool(name="dram", bufs=1, space="DRAM") as dram,
        ):
            # constants
            W1sb = cst.tile([FIN, HID], f32)
            nc.sync.dma_start(W1sb[:], W1[:])
            W1h = cst.tile([FIN, HID], f16)
            nc.vector.tensor_copy(out=W1h[:], in_=W1sb[:])
            W2sb = cst.tile([HID, HID], f32)
            nc.sync.dma_start(W2sb[:], W2[:])
            W2h = cst.tile([HID, HID], f16)
            nc.vector.tensor_copy(out=W2h[:], in_=W2sb[:])
            Wcb = cst.tile([P, HID], f16)
            nc.sync.dma_start(Wcb[:], WcBC[:])
            eyesb = cst.tile([P, P], f16)
            nc.sync.dma_start(eyesb[:], eye[:])
            iotasb = cst.tile([P, P], f16)
            nc.sync.dma_start(iotasb[:], iota[:])
            dst_sb = cst.tile([P, tts], f16)
            nc.sync.dma_start(dst_sb[:], dstrel[:])
            dBsb = cst.tile([P, NBLK], f32)
            nc.sync.dma_start(dBsb[:], dinvB[:])
            if has_b1:
                b1sb = cst.tile([P, HID], f32)
                nc.sync.dma_start(b1sb[:], b1bc[:])
            if has_b2:
                b2sb = cst.tile([P, HID], f32)
                nc.sync.dma_start(b2sb[:], b2bc[:])
            if has_bc:
                bcsb = cst.tile([1, 1], f32)
                nc.sync.dma_start(bcsb[:], bc[:])

            hs1f = [dram.tile([CHPAD, HID], f16, name=f"hs1f{c}")
                    for c in range(NCHUNK)]
            hs2s = dram.tile([NLB, HID], f16)
            hs2f = [dram.tile([CHUNK, HID], f16, addr_space="Shared",
                              name=f"hs2f{c}") for c in range(NCHUNK)]

            # ---- phase A: hs1 = dinv * (x @ W1) for ALL nodes, fp16 packed
            for g in range(NG):
                xb = io.tile([FIN, GR * P], f16, name="xb")
                nc.sync.dma_start(xb[:], xT[:, g * GR * P:(g + 1) * GR * P])
                ps = psA.tile([P, GR, HID], f32, name="psA")
                for j in range(GR):
                    nc.tensor.matmul(out=ps[:, j, :],
                                     lhsT=xb[:, j * P:(j + 1) * P],
                                     rhs=W1h[:], start=True, stop=True)
                hsg = work.tile([P, GR, HID], f16, name="hsg")
                nc.scalar.activation(out=hsg[:], in_=ps[:], func=copy_)
                gl = (g % NGC) * GR * P
                dst1 = hs1f[g // NGC][gl:gl + GR * P, :]
                if QUAD:
                    nc.sync.dma_start(
                        dst1.rearrange("(u p r) f -> p u (r f)", u=2, p=P),
                        hsg[:].rearrange("p (u q) f -> p u (q f)", u=2))
                else:
                    nc.sync.dma_start(
                        dst1.rearrange("(j p) f -> p j f", p=P),
                        hsg[:])

            # stream column/idx offsets per (sb, chunk)
            col_off = np.zeros((NSB, NCHUNK), np.int64)
            ixw_off = np.zeros((NSB, NCHUNK), np.int64)
            acc_to, acc_ti = 0, 0
            for sb in range(NSB):
                for c in range(NCHUNK):
                    col_off[sb, c] = acc_to
                    ixw_off[sb, c] = acc_ti
                    acc_to += int(seg_scols[sb][c])
                    acc_ti += int(seg_tiles[sb][c]) * P // 16

            def seg_load(table, sb, c):
                st = int(seg_tiles[sb][c])
                sc = int(seg_scols[sb][c])
                n_idx = st * P
                io_ = int(ixw_off[sb, c])
                to_ = int(col_off[sb, c])
                ix = io.tile([P, n_idx // 16], i16, name="ix")
                nc.sync.dma_start(ix[:], idx[:, io_:io_ + n_idx // 16])
                mg = msgp.tile([P, st, P], f16, name="mg")
                nc.gpsimd.dma_gather(
                    mg[:],
                    table.rearrange("(q two) f -> q (two f)", two=2),
                    ix[:], n_idx, n_idx, P, single_packet=False)
                St = sp.tile([P, sc, BS], f16, name="St")
                nc.vector.tensor_tensor(
                    out=St[:],
                    in0=dst_sb[:, to_:to_ + sc, None]
                        .to_broadcast([P, sc, BS]),
                    in1=iotasb[:, None, :BS]
                        .to_broadcast([P, sc, BS]),
                    op=mybir.AluOpType.is_equal)
                return mg, St

            def block_mms(pt, b, msgs, Ss, chunks, start, stop):
                mms = [e for e in mms_all[b] if e[0] in chunks]
                for j, (c, t, sc, par) in enumerate(mms):
                    nc.tensor.matmul(
                        out=pt[:],
                        lhsT=Ss[c][:, sc, :],
                        rhs=msgs[c][:, t, par * HID:(par + 1) * HID],
                        start=start and (j == 0),
                        stop=stop and (j == len(mms) - 1))

            def scale_relu(pt_or_t, b, has_b, bsb):
                """h = relu(dinv*x + b) as fp16 [BS, HID]."""
                hL = work.tile([BS, HID], f16, name="hL")
                if has_b:
                    tsc = work.tile([BS, HID], f32, name="tsc")
                    nc.scalar.activation(out=tsc[:], in_=pt_or_t[:],
                                         func=copy_,
                                         scale=dBsb[:BS, b:b + 1])
                    tbb = work.tile([BS, HID], f32, name="tbb")
                    nc.vector.tensor_tensor(out=tbb[:], in0=tsc[:],
                                            in1=bsb[:BS, :],
                                            op=mybir.AluOpType.add)
                    nc.scalar.activation(out=hL[:], in_=tbb[:], func=relu)
                else:
                    nc.scalar.activation(out=hL[:], in_=pt_or_t[:],
                                         func=relu,
                                         scale=dBsb[:BS, b:b + 1])
                return hL

            half = NLOC // NCHUNK
            ag_after_sb = {}
            for c in range(NCHUNK - 1):
                # AG#c ins ready once blocks covering rows < (c+1)*half done
                blk_ready = ((c + 1) * half + BS - 1) // BS
                ag_after_sb[min((blk_ready - 1) // SBB + 1, NSB - 1)] = c

            # ---- layer 1: one pass over both chunks, epilogue -> hs2s
            for sb in range(NSB):
                blk_lo, blk_hi = sb * SBB, min((sb + 1) * SBB, NBLK)
                msgs, Ss = [], []
                for c in range(NCHUNK):
                    mg, St = seg_load(hs1f[c][0:CHUNK, :], sb, c)
                    msgs.append(mg)
                    Ss.append(St)
                for b in range(blk_lo, blk_hi):
                    pt = aggp.tile([BS, HID], f32, name="pt")
                    block_mms(pt, b, msgs, Ss, tuple(range(NCHUNK)), True, True)
                    hL = scale_relu(pt, b, has_b1, b1sb if has_b1 else None)
                    # hs2 = dinv * (h1 @ W2): transpose h1 then matmul
                    ptr = ptrp.tile([HID, BS], f16, name="ptr")
                    nc.tensor.transpose(ptr[:], hL[:], eyesb[:BS, :BS])
                    h1T = work.tile([HID, BS], f16, name="h1T")
                    nc.scalar.activation(out=h1T[:], in_=ptr[:], func=copy_)
                    ps2 = p2p.tile([BS, HID], f32, name="ps2")
                    nc.tensor.matmul(out=ps2[:], lhsT=h1T[:],
                                     rhs=W2h[:], start=True, stop=True)
                    h2r = work.tile([BS, HID], f16, name="h2r")
                    nc.scalar.activation(out=h2r[:], in_=ps2[:],
                                         func=copy_,
                                         scale=dBsb[:BS, b:b + 1])
                    nc.sync.dma_start(hs2s[b * BS:(b + 1) * BS, :], h2r[:])
                if sb in ag_after_sb:
                    c = ag_after_sb[sb]
                    nc.gpsimd.collective_compute(
                        "AllGather", mybir.AluOpType.bypass,
                        replica_groups=[list(range(NC_))],
                        ins=[hs2s[c * half:(c + 1) * half, :]],
                        outs=[hs2f[c][:]],
                    )
            nc.gpsimd.collective_compute(
                "AllGather", mybir.AluOpType.bypass,
                replica_groups=[list(range(NC_))],
                ins=[hs2s[(NCHUNK - 1) * half:NCHUNK * half, :]],
                outs=[hs2f[NCHUNK - 1][:]],
            )

            # ---- layer 2: one pass per chunk, partials held in SBUF accs
            # (ping-pong buffers: pass p writes acc[p%2] reading acc[1-p%2])
            accs = [accp.tile([P, NBLK, HID], f16, name=f"acc{i}")
                    for i in range(2)]
            for cpass in range(NCHUNK):
                last = cpass == NCHUNK - 1
                awr = accs[cpass % 2]
                ard = accs[1 - cpass % 2]
                for sb in range(NSB):
                    blk_lo, blk_hi = sb * SBB, min((sb + 1) * SBB, NBLK)
                    mg, St = seg_load(hs2f[cpass][:], sb, cpass)
                    for b in range(blk_lo, blk_hi):
                        pt = aggp.tile([BS, HID], f32, name="pt")
                        block_mms(pt, b, {cpass: mg}, {cpass: St},
                                  (cpass,), True, True)
                        if not last:
                            if cpass == 0:
                                nc.scalar.activation(out=awr[:BS, b, :],
                                                     in_=pt[:], func=copy_)
                            else:
                                nc.vector.tensor_tensor(
                                    out=awr[:BS, b, :], in0=pt[:],
                                    in1=ard[:BS, b, :],
                                    op=mybir.AluOpType.add)
                            continue
                        t2 = work.tile([BS, HID], f32, name="t2")
                        nc.vector.tensor_tensor(out=t2[:], in0=pt[:],
                                                in1=ard[:BS, b, :],
                                                op=mybir.AluOpType.add)
                        hL = scale_relu(t2, b, has_b2,
                                        b2sb if has_b2 else None)
                        # head: out = relu(h2) @ Wc (+ bc); the free-axis sum
                        # rides the Act accumulator, the mult runs on Pool
                        mwc = work.tile([BS, HID], f32, name="mwc")
                        nc.gpsimd.tensor_mul(mwc[:], hL[:], Wcb[:BS, :])
                        mwd = work.tile([BS, HID], f32, name="mwd")
                        oc = work.tile([BS, 1], f32, name="oc")
                        nc.scalar.activation(out=mwd[:], in_=mwc[:],
                                             func=copy_, accum_out=oc[:])
                        if has_bc:
                            oc2 = work.tile([BS, 1], f32, name="oc2")
                            nc.vector.tensor_scalar(
                                out=oc2[:], in0=oc[:],
                                scalar1=bcsb[:1, :1], scalar2=None,
                                op0=mybir.AluOpType.add)
                            oc = oc2
                        nc.sync.dma_start(
                            out[b * BS:(b + 1) * BS, :], oc[:])

    nc.compile()
    return nc


_CACHE = {}


def kernel(x, edge_index, W1, b1, W2, b2, Wc, bc):
    x = np.asarray(x, np.float32)
    edge_index = np.asarray(edge_index, np.int32)
    (in_maps, nt, seg_tiles, seg_scols, st_max, sc_max, tt, tts,
     mms_all) = _prep(x, edge_index)

    b1 = np.asarray(b1, np.float32).reshape(-1)
    b2 = np.asarray(b2, np.float32).reshape(-1)
    bc = np.asarray(bc, np.float32).reshape(-1)
    Wc = np.asarray(Wc, np.float32).reshape(-1)
    has_b1 = bool(np.any(b1 != 0))
    has_b2 = bool(np.any(b2 != 0))
    has_bc = bool(np.any(bc != 0))

    key = (st_max, tt, tts, nt.tobytes(),
           tuple(np.asarray(seg_scols).ravel()), has_b1, has_b2, has_bc)
    if key not in _CACHE:
        _CACHE[key] = _build(seg_tiles, seg_scols, st_max, sc_max, tt, tts,
                             mms_all, has_b1, has_b2, has_bc)
    nc = _CACHE[key]

    shared = {
        "W1": np.asarray(W1, np.float32),
        "W2": np.asarray(W2, np.float32),
        "WcBC": np.tile(Wc.astype(np.float16), (P, 1)),
        "b1bc": np.tile(b1, (P, 1)),
        "b2bc": np.tile(b2, (P, 1)),
        "bc": bc.reshape(1, 1),
    }
    for m in in_maps:
        m.update(shared)

    res = run_bass_kernel_spmd(nc, in_maps, core_ids=list(range(NC_)))
    return np.concatenate(
        [res.results[k]["out"][:NLOC, 0] for k in range(NC_)]
    ).astype(np.float32)
